# revision 3
# baseline (speedup 1.0000x reference)
"""Multi-head attention (B=2, S=2048, H=768, NH=12) on 8 TRN2 NeuronCores.

Sharding: core c handles batch b = c//4 and head group g = c%4 (3 heads of 64).
Per-core device kernel computes, entirely in transposed "feature-major" layouts:
  qT/kT/vT = W.T-slice @ x.T  (contraction over H on partitions)
  scoresT[k,q] = kT.T-slices @ qT  (per 128-wide k tile)
  expT = exp(scoresT/8 + mask[k])  (mask folded in as per-partition ACT bias;
                                    no max-subtraction: scores are O(6))
  ctxT[d,q] += v_nat[k,d].T-tiles @ expT   with v augmented by a ones column so
  row 64 of ctxT accumulates the softmax denominator. A final PE transpose per
  128-q block yields natural layout where the denominator is a per-partition
  scalar: out = ctx * (1/denom) via DVE, then DMA to the [2048, 192] output.

All matmuls run in float32r (fp32 bits, reduced-precision PE mode, full rate at
N>=256; measured ~1.5e-4 matmul rel err).

Schedule: Q then K projections accumulate chunk-by-chunk into 8 PSUM banks
(ic-outer) so the input DMA stream never stalls; attention (ACT exp is the
critical engine, ~101us busy) starts as soon as kT is ready (~37us). The V
projection runs under the attention phase with a 2-bank PSUM pool, m0 (heads
0,1) first so ctx matmuls can drain the expT backlog early.
"""

import numpy as np

import concourse.bass as bass
import concourse.bacc as bacc
import concourse.mybir as mybir
import concourse.tile as tile
from concourse.bass_utils import run_bass_kernel_spmd

F32R = mybir.dt.float32r
F32 = mybir.dt.float32

B, S, H, NH, D = 2, 2048, 768, 12, 64
NCORES = 8
GH = 3               # heads per core
GD = GH * D          # 192 features per core
NIC = H // 128       # 6 contraction chunks
NKT = S // 128       # 16 k-tiles
NQH = 2              # q halves per head
QW = S // NQH        # 1024
SCALE = 1.0 / 8.0    # 1/sqrt(D)


def _build():
    nc = bacc.Bacc("TRN2", target_bir_lowering=False)

    xT = {n: nc.dram_tensor(f"x{n}T", [H, S], F32R, kind="ExternalInput")
          for n in "qkv"}
    w = {n: nc.dram_tensor(f"w{n}", [H, GD], F32R, kind="ExternalInput")
         for n in "qkv"}
    bias = {n: nc.dram_tensor(f"b{n}", [GD], F32, kind="ExternalInput")
            for n in "qkv"}
    maskT = nc.dram_tensor("maskT", [128, NKT], F32, kind="ExternalInput")
    idents = nc.dram_tensor("idents", [128, 192], F32R, kind="ExternalInput")
    out = nc.dram_tensor("out", [S, GD], F32, kind="ExternalOutput")

    with tile.TileContext(nc) as tc:
        with tc.tile_pool(name="singles", bufs=1) as singles, \
             tc.tile_pool(name="wpool", bufs=2) as wpool, \
             tc.tile_pool(name="xin", bufs=6) as xin, \
             tc.tile_pool(name="ex", bufs=14) as expool, \
             tc.tile_pool(name="cx", bufs=2) as cxpool, \
             tc.tile_pool(name="osb", bufs=4) as osb:

            ident = singles.tile([128, 192], F32R)
            nc.sync.dma_start(ident[:], idents[:])
            mask_sb = singles.tile([128, NKT], F32)
            nc.sync.dma_start(mask_sb[:], maskT[:])

            w_sb, b_sb = {}, {}
            for n in "qkv":
                w_sb[n] = wpool.tile([128, NIC, GD], F32R, tag="w", name=f"w_{n}")
                nc.sync.dma_start(w_sb[n][:], w[n].rearrange("(c p) d -> p c d", p=128))
                b_sb[n] = singles.tile([128, 2], F32, name=f"b_{n}")
                # col 0: bias rows 0:128 (m0); col 1: bias rows 128:192 in rows 0:64
                nc.sync.dma_start(b_sb[n][:, 0:1],
                                  bias[n][0:128].rearrange("(p o) -> p o", o=1))
                nc.sync.dma_start(b_sb[n][0:64, 1:2],
                                  bias[n][128:192].rearrange("(p o) -> p o", o=1))

            # pT[n]: projection outputs, feature-major (m0: heads 0,1; m1: head 2)
            pT = {n: (singles.tile([128, S], F32R, name=f"pT_{n}0"),
                      singles.tile([64, S], F32R, name=f"pT_{n}1"))
                  for n in "qkv"}
            MT = ((128, 0), (64, 128))  # (rows, feature offset) per m tile

            def load_chunks(n):
                ch = [xin.tile([128, S], F32R, tag="x", name=f"x_{n}{ic}")
                      for ic in range(NIC)]
                for ic in range(NIC):
                    nc.sync.dma_start(ch[ic][:], xT[n][bass.ts(ic, 128), :])
                return ch

            def evict(n, m, nb, width, acc):
                rows = MT[m][0]
                nc.vector.tensor_scalar_add(
                    pT[n][m][:, bass.ds(nb * width, width)], acc[:],
                    b_sb[n][0:rows, m:m + 1])

            # ---- phase 1: Q then K, ic-outer into an 8-bank PSUM pool
            with tc.tile_pool(name="pproj", bufs=8, space="PSUM") as pproj:
                for n in "qk":
                    x_ch = load_chunks(n)
                    accs = {}
                    for m, (rows, dlo) in enumerate(MT):
                        for nb in range(4):
                            accs[m, nb] = pproj.tile([rows, 512], F32, tag="pg",
                                                     name=f"pg_{n}{m}{nb}")
                    for ic in range(NIC):
                        for (m, nb), acc in accs.items():
                            rows, dlo = MT[m]
                            nc.tensor.matmul(
                                acc[:], w_sb[n][:, ic, dlo:dlo + rows],
                                x_ch[ic][:, bass.ts(nb, 512)],
                                start=(ic == 0), stop=(ic == NIC - 1))
                    for (m, nb), acc in accs.items():
                        evict(n, m, nb, 512, acc)

            # ---- phase 2: V projection + attention share the 8 banks (2+4+2)
            with tc.tile_pool(name="pmisc", bufs=2, space="PSUM") as pmisc, \
                 tc.tile_pool(name="psc", bufs=2, space="PSUM") as psc, \
                 tc.tile_pool(name="pctx", bufs=1, space="PSUM") as pctx:

                v_nat = singles.tile([128, GH, NKT, D + 2], F32R)
                ones_f = singles.tile([128, 2], F32)
                nc.vector.memset(ones_f[:, 0:1], 1.0)
                nc.vector.memset(ones_f[:, 1:2], 0.0)
                for h in range(GH):
                    nc.vector.tensor_copy(
                        v_nat[:, h, :, D:D + 2],
                        ones_f.rearrange("p (k o) -> p k o", k=1)
                        .broadcast_to([128, NKT, 2]))

                def head_ops(h):
                    if h == 0:
                        return pT["q"][0][0:64], pT["k"][0][0:64], \
                            pT["v"][0][0:64], ident[0:64, 0:64]
                    if h == 1:
                        return pT["q"][0][64:128], pT["k"][0][64:128], \
                            pT["v"][0][64:128], ident[64:128, 128:192]
                    return pT["q"][1], pT["k"][1], pT["v"][1], ident[0:64, 0:64]

                x_ch = load_chunks("v")

                def v_proj_m(m):
                    rows, dlo = MT[m]
                    for nb in range(4):
                        acc = pmisc.tile([rows, 512], F32, tag="mm",
                                         name=f"vpg{m}{nb}")
                        for ic in range(NIC):
                            nc.tensor.matmul(
                                acc[:], w_sb["v"][:, ic, dlo:dlo + rows],
                                x_ch[ic][:, bass.ts(nb, 512)],
                                start=(ic == 0), stop=(ic == NIC - 1))
                        evict("v", m, nb, 512, acc)

                def v_transposes(h):
                    _, _, vTh, idh = head_ops(h)
                    for kt in range(NKT):
                        tp = pmisc.tile([128, D], F32R, tag="mm", name="tp")
                        nc.tensor.transpose(tp[:], vTh[:, bass.ts(kt, 128)], idh)
                        nc.vector.tensor_copy(v_nat[:, h, kt, 0:D], tp[:])

                v_proj_m(0)
                v_transposes(0)
                v_transposes(1)
                v_proj_m(1)
                v_transposes(2)

                # ---- attention
                for h in range(GH):
                    qTh, kTh, _, _ = head_ops(h)
                    for qh in range(NQH):
                        ctx = pctx.tile([D + 2, QW], F32, name="ctx")
                        for kt in range(NKT):
                            sc = psc.tile([128, QW], F32, name="sc")
                            for j in range(QW // 512):
                                nc.tensor.matmul(
                                    sc[:, bass.ts(j, 512)],
                                    kTh[:, bass.ts(kt, 128)],
                                    qTh[:, bass.ds(qh * QW + j * 512, 512)],
                                    start=True, stop=True)
                            ex = expool.tile([128, QW], F32R, name="ex")
                            nc.scalar.activation(
                                ex[:], sc[:], mybir.ActivationFunctionType.Exp,
                                bias=mask_sb[:, kt:kt + 1], scale=SCALE)
                            for j in range(QW // 512):
                                nc.tensor.matmul(
                                    ctx[:, bass.ts(j, 512)],
                                    v_nat[:, h, kt, :],
                                    ex[:, bass.ts(j, 512)],
                                    start=(kt == 0), stop=(kt == NKT - 1))
                        ctxT = cxpool.tile([D + 2, QW], F32R, name="ctxT")
                        nc.vector.tensor_copy(ctxT[:], ctx[:])
                        for qb in range(QW // 128):
                            cn = pmisc.tile([128, D + 2], F32R, tag="mm", name="cn")
                            nc.tensor.transpose(cn[:], ctxT[:, bass.ts(qb, 128)],
                                                ident[0:D + 2, 0:D + 2])
                            rec = osb.tile([128, 1], F32, tag="rec", name="rec")
                            nc.vector.reciprocal(rec[:], cn[:, D:D + 1])
                            o = osb.tile([128, D], F32, tag="o", name="o")
                            nc.vector.tensor_scalar_mul(o[:], cn[:, 0:D], rec[:])
                            nc.sync.dma_start(
                                out[bass.ds(qh * QW + qb * 128, 128),
                                    bass.ds(h * D, D)],
                                o[:])
    nc.finalize()
    return nc


_NC_CACHE = []


def _get_nc():
    if not _NC_CACHE:
        _NC_CACHE.append(_build())
    return _NC_CACHE[0]


def _idents_np():
    idents = np.zeros((128, 192), np.float32)
    idents[:, :128] = np.eye(128, dtype=np.float32)
    idents[64:128, 128:192] = np.eye(64, dtype=np.float32)
    return idents


def _in_maps(query, key, value, attention_mask, Wq, bq, Wk, bk, Wv, bv):
    query = np.asarray(query, np.float32)
    key = np.asarray(key, np.float32)
    value = np.asarray(value, np.float32)
    attention_mask = np.asarray(attention_mask, np.float32)
    ws = {"q": np.asarray(Wq, np.float32), "k": np.asarray(Wk, np.float32),
          "v": np.asarray(Wv, np.float32)}
    bs = {"q": np.asarray(bq, np.float32), "k": np.asarray(bk, np.float32),
          "v": np.asarray(bv, np.float32)}
    xs = {"q": query, "k": key, "v": value}
    idents = _idents_np()
    maps = []
    for c in range(NCORES):
        b, g = divmod(c, NCORES // B)
        sl = slice(g * GD, (g + 1) * GD)
        m = {}
        for n in "qkv":
            m[f"x{n}T"] = np.ascontiguousarray(xs[n][b].T)
            m[f"w{n}"] = np.ascontiguousarray(ws[n][sl].T)
            m[f"b{n}"] = np.ascontiguousarray(bs[n][sl])
        m["maskT"] = np.ascontiguousarray(
            attention_mask[b, 0, 0].reshape(NKT, 128).T)
        m["idents"] = idents
        maps.append(m)
    return maps


def _run(in_maps, **kw):
    return run_bass_kernel_spmd(_get_nc(), in_maps, core_ids=list(range(NCORES)), **kw)


def _assemble(results):
    full = np.empty((B, S, H), np.float32)
    for c, res in enumerate(results):
        b, g = divmod(c, NCORES // B)
        full[b, :, g * GD:(g + 1) * GD] = res["out"]
    return full


def kernel(query, key, value, attention_mask, Wq, bq, Wk, bk, Wv, bv):
    maps = _in_maps(query, key, value, attention_mask, Wq, bq, Wk, bk, Wv, bv)
    r = _run(maps)
    return _assemble(r.results)


# revision 7
# speedup vs baseline: 1.3704x; 1.3704x over previous
"""Multi-head attention (B=2, S=2048, H=768, NH=12) on 8 TRN2 NeuronCores.

Sharding: core c handles batch b = c//4 and head group g = c%4 (3 heads of 64).
Per-core device kernel computes, entirely in transposed "feature-major" layouts:
  qT/kT/vT = W.T-slice @ x.T  (contraction over H on partitions)
  scoresT[k,q] = kT.T-slices @ qT  (per 128-wide k tile)
  expT = exp(scoresT/8 + mask[k])  (mask folded in as per-partition ACT bias;
                                    no max-subtraction: scores are O(6))
  ctxT[d,q] += v_nat[k,d].T-tiles @ expT   with v augmented by a ones column so
  row 64 of ctxT accumulates the softmax denominator. A final PE transpose per
  128-q block yields natural layout where the denominator is a per-partition
  scalar: out = ctx * (1/denom) via DVE, then DMA to the [2048, 192] output.

All matmuls run in float32r (fp32 bits, reduced-precision PE mode, full rate at
N>=256; measured ~1.5e-4 matmul rel err).

Schedule: Q then K projections accumulate chunk-by-chunk into 8 PSUM banks
(ic-outer) so the input DMA stream never stalls; attention (ACT exp is the
critical engine, ~101us busy) starts as soon as kT is ready (~37us). The V
projection runs under the attention phase with a 2-bank PSUM pool, m0 (heads
0,1) first so ctx matmuls can drain the expT backlog early.
"""

import numpy as np

import concourse.bass as bass
import concourse.bacc as bacc
import concourse.mybir as mybir
import concourse.tile as tile
from concourse.bass_utils import run_bass_kernel_spmd

F32R = mybir.dt.float32r
F32 = mybir.dt.float32

B, S, H, NH, D = 2, 2048, 768, 12, 64
NCORES = 8
GH = 3               # heads per core
GD = GH * D          # 192 features per core
NIC = H // 128       # 6 contraction chunks
NKT = S // 128       # 16 k-tiles
NQH = 2              # q halves per head
QW = S // NQH        # 1024
SCALE = 1.0 / 8.0    # 1/sqrt(D)


def _build():
    nc = bacc.Bacc("TRN2", target_bir_lowering=False)

    xT = {n: nc.dram_tensor(f"x{n}T", [H, S], F32R, kind="ExternalInput")
          for n in "qkv"}
    w = {n: nc.dram_tensor(f"w{n}", [H, GD], F32R, kind="ExternalInput")
         for n in "qkv"}
    bias = {n: nc.dram_tensor(f"b{n}", [GD], F32, kind="ExternalInput")
            for n in "qkv"}
    maskT = nc.dram_tensor("maskT", [128, NKT], F32, kind="ExternalInput")
    idents = nc.dram_tensor("idents", [128, 192], F32R, kind="ExternalInput")
    out = nc.dram_tensor("out", [S, GD], F32, kind="ExternalOutput")

    with tile.TileContext(nc) as tc:
        with tc.tile_pool(name="singles", bufs=1) as singles, \
             tc.tile_pool(name="wpool", bufs=2) as wpool, \
             tc.tile_pool(name="xin", bufs=6) as xin, \
             tc.tile_pool(name="ex", bufs=14) as expool, \
             tc.tile_pool(name="cx", bufs=2) as cxpool, \
             tc.tile_pool(name="osb", bufs=4) as osb, \
             tc.tile_pool(name="ost", bufs=1) as ost:

            ident = singles.tile([128, 192], F32R)
            nc.sync.dma_start(ident[:], idents[:])
            mask_sb = singles.tile([128, NKT], F32)
            nc.sync.dma_start(mask_sb[:], maskT[:])

            w_sb, b_sb = {}, {}
            for n in "qkv":
                w_sb[n] = wpool.tile([128, NIC, GD], F32R, tag="w", name=f"w_{n}")
                nc.sync.dma_start(w_sb[n][:], w[n].rearrange("(c p) d -> p c d", p=128))
                b_sb[n] = singles.tile([128, 2], F32, name=f"b_{n}")
                # col 0: bias rows 0:128 (m0); col 1: bias rows 128:192 in rows 0:64
                nc.sync.dma_start(b_sb[n][:, 0:1],
                                  bias[n][0:128].rearrange("(p o) -> p o", o=1))
                nc.sync.dma_start(b_sb[n][0:64, 1:2],
                                  bias[n][128:192].rearrange("(p o) -> p o", o=1))

            # pT[n]: projection outputs, feature-major (m0: heads 0,1; m1: head 2)
            pT = {n: (singles.tile([128, S], F32R, name=f"pT_{n}0"),
                      singles.tile([64, S], F32R, name=f"pT_{n}1"))
                  for n in "qkv"}
            MT = ((128, 0), (64, 128))  # (rows, feature offset) per m tile

            def load_chunks(n):
                ch = [xin.tile([128, S], F32R, tag="x", name=f"x_{n}{ic}")
                      for ic in range(NIC)]
                for ic in range(NIC):
                    nc.sync.dma_start(ch[ic][:], xT[n][bass.ts(ic, 128), :])
                return ch

            def evict(n, m, nb, width, acc):
                rows = MT[m][0]
                nc.vector.tensor_scalar_add(
                    pT[n][m][:, bass.ds(nb * width, width)], acc[:],
                    b_sb[n][0:rows, m:m + 1])

            # ---- phase 1: Q then K, ic-outer into an 8-bank PSUM pool
            with tc.tile_pool(name="pproj", bufs=8, space="PSUM") as pproj:
                for n in "qk":
                    x_ch = load_chunks(n)
                    accs = {}
                    for m, (rows, dlo) in enumerate(MT):
                        for nb in range(4):
                            accs[m, nb] = pproj.tile([rows, 512], F32, tag="pg",
                                                     name=f"pg_{n}{m}{nb}")
                    for ic in range(NIC):
                        for (m, nb), acc in accs.items():
                            rows, dlo = MT[m]
                            nc.tensor.matmul(
                                acc[:], w_sb[n][:, ic, dlo:dlo + rows],
                                x_ch[ic][:, bass.ts(nb, 512)],
                                start=(ic == 0), stop=(ic == NIC - 1))
                    for (m, nb), acc in accs.items():
                        evict(n, m, nb, 512, acc)

            # ---- phase 2: V projection + attention share the 8 banks (2+4+2)
            with tc.tile_pool(name="pmisc", bufs=2, space="PSUM") as pmisc, \
                 tc.tile_pool(name="psc", bufs=2, space="PSUM") as psc, \
                 tc.tile_pool(name="pctx", bufs=1, space="PSUM") as pctx:

                v_nat = singles.tile([128, GH, NKT, D + 2], F32R)
                ones_f = singles.tile([128, 2], F32)
                nc.vector.memset(ones_f[:, 0:1], 1.0)
                nc.vector.memset(ones_f[:, 1:2], 0.0)
                for h in range(GH):
                    nc.vector.tensor_copy(
                        v_nat[:, h, :, D:D + 2],
                        ones_f.rearrange("p (k o) -> p k o", k=1)
                        .broadcast_to([128, NKT, 2]))

                def head_ops(h):
                    if h == 0:
                        return pT["q"][0][0:64], pT["k"][0][0:64], \
                            pT["v"][0][0:64], ident[0:64, 0:64]
                    if h == 1:
                        return pT["q"][0][64:128], pT["k"][0][64:128], \
                            pT["v"][0][64:128], ident[64:128, 128:192]
                    return pT["q"][1], pT["k"][1], pT["v"][1], ident[0:64, 0:64]

                out_sb = [ost.tile([128, GD], F32, tag=f"ost{qb}", name=f"out{qb}")
                          for qb in range(S // 128)]

                x_ch = load_chunks("v")

                def v_proj_m(m):
                    rows, dlo = MT[m]
                    for nb in range(4):
                        acc = pmisc.tile([rows, 512], F32, tag="mm",
                                         name=f"vpg{m}{nb}")
                        for ic in range(NIC):
                            nc.tensor.matmul(
                                acc[:], w_sb["v"][:, ic, dlo:dlo + rows],
                                x_ch[ic][:, bass.ts(nb, 512)],
                                start=(ic == 0), stop=(ic == NIC - 1))
                        evict("v", m, nb, 512, acc)

                def v_transposes(h):
                    _, _, vTh, idh = head_ops(h)
                    for kt in range(NKT):
                        tp = pmisc.tile([128, D], F32R, tag="mm", name="tp")
                        nc.tensor.transpose(tp[:], vTh[:, bass.ts(kt, 128)], idh)
                        nc.vector.tensor_copy(v_nat[:, h, kt, 0:D], tp[:])

                v_proj_m(0)
                v_transposes(0)
                v_transposes(1)
                v_proj_m(1)
                v_transposes(2)

                # ---- attention
                for h in range(GH):
                    qTh, kTh, _, _ = head_ops(h)
                    for qh in range(NQH):
                        ctx = pctx.tile([D + 2, QW], F32, name="ctx")
                        for kt in range(NKT):
                            sc = psc.tile([128, QW], F32, name="sc")
                            for j in range(QW // 512):
                                nc.tensor.matmul(
                                    sc[:, bass.ts(j, 512)],
                                    kTh[:, bass.ts(kt, 128)],
                                    qTh[:, bass.ds(qh * QW + j * 512, 512)],
                                    start=True, stop=True)
                            ex = expool.tile([128, QW], F32R, name="ex")
                            nc.scalar.activation(
                                ex[:], sc[:], mybir.ActivationFunctionType.Exp,
                                bias=mask_sb[:, kt:kt + 1], scale=SCALE)
                            for j in range(QW // 512):
                                nc.tensor.matmul(
                                    ctx[:, bass.ts(j, 512)],
                                    v_nat[:, h, kt, :],
                                    ex[:, bass.ts(j, 512)],
                                    start=(kt == 0), stop=(kt == NKT - 1))
                        ctxT = cxpool.tile([D + 2, QW], F32R, name="ctxT")
                        nc.vector.tensor_copy(ctxT[:], ctx[:])
                        for qb in range(QW // 128):
                            cn = pmisc.tile([128, D + 2], F32R, tag="mm", name="cn")
                            nc.tensor.transpose(cn[:], ctxT[:, bass.ts(qb, 128)],
                                                ident[0:D + 2, 0:D + 2])
                            rec = osb.tile([128, 1], F32, tag="rec", name="rec")
                            nc.vector.reciprocal(rec[:], cn[:, D:D + 1])
                            gqb = qh * (QW // 128) + qb
                            nc.vector.tensor_scalar_mul(
                                out_sb[gqb][:, bass.ds(h * D, D)],
                                cn[:, 0:D], rec[:])
                            if h == GH - 1:
                                nc.sync.dma_start(
                                    out[bass.ds(gqb * 128, 128), :],
                                    out_sb[gqb][:])
    nc.finalize()
    return nc


_NC_CACHE = []


def _get_nc():
    if not _NC_CACHE:
        _NC_CACHE.append(_build())
    return _NC_CACHE[0]


def _idents_np():
    idents = np.zeros((128, 192), np.float32)
    idents[:, :128] = np.eye(128, dtype=np.float32)
    idents[64:128, 128:192] = np.eye(64, dtype=np.float32)
    return idents


def _in_maps(query, key, value, attention_mask, Wq, bq, Wk, bk, Wv, bv):
    query = np.asarray(query, np.float32)
    key = np.asarray(key, np.float32)
    value = np.asarray(value, np.float32)
    attention_mask = np.asarray(attention_mask, np.float32)
    ws = {"q": np.asarray(Wq, np.float32), "k": np.asarray(Wk, np.float32),
          "v": np.asarray(Wv, np.float32)}
    bs = {"q": np.asarray(bq, np.float32), "k": np.asarray(bk, np.float32),
          "v": np.asarray(bv, np.float32)}
    xs = {"q": query, "k": key, "v": value}
    idents = _idents_np()
    maps = []
    for c in range(NCORES):
        b, g = divmod(c, NCORES // B)
        sl = slice(g * GD, (g + 1) * GD)
        m = {}
        for n in "qkv":
            m[f"x{n}T"] = np.ascontiguousarray(xs[n][b].T)
            m[f"w{n}"] = np.ascontiguousarray(ws[n][sl].T)
            m[f"b{n}"] = np.ascontiguousarray(bs[n][sl])
        m["maskT"] = np.ascontiguousarray(
            attention_mask[b, 0, 0].reshape(NKT, 128).T)
        m["idents"] = idents
        maps.append(m)
    return maps


def _run(in_maps, **kw):
    return run_bass_kernel_spmd(_get_nc(), in_maps, core_ids=list(range(NCORES)), **kw)


def _assemble(results):
    full = np.empty((B, S, H), np.float32)
    for c, res in enumerate(results):
        b, g = divmod(c, NCORES // B)
        full[b, :, g * GD:(g + 1) * GD] = res["out"]
    return full


def kernel(query, key, value, attention_mask, Wq, bq, Wk, bk, Wv, bv):
    maps = _in_maps(query, key, value, attention_mask, Wq, bq, Wk, bk, Wv, bv)
    r = _run(maps)
    return _assemble(r.results)
